# revision 13
# baseline (speedup 1.0000x reference)
"""Trainium2 Bass kernel for nn_RankNetFast_2491081031806.

Math: with t = target[random_docs], s = scores[random_docs, 0],
  lam_ij = -0.5*sign(t_i - t_j);  row_i = sum_j lam_ij
  loss = sum(cumsum(row) * s) = sum_i row_i * T_i,  T = inclusive suffix sums of s.
Since t takes values {0..4}, row_i = -0.5 * sum_v c_v * sign(t_i - v) where c_v is
the global histogram of t. So the kernel is: gather 8192 (s, t) pairs, histogram,
suffix-sum, weighted dot -- no 8192^2 matrix.

Device layout: the 8192 samples sit in a [128, 64] tile, sample k at
(p, j) = (k % 128, k // 128), so the inclusive suffix sum over k is
  T = Lincl @ s  (within-column suffix via matmul)  +  colsum-tail (matmul)
All reductions across partitions are matmuls with ones/triangular constants.

Sharded variant: core m gathers samples [m*1024, (m+1)*1024) (columns
j in [m*8, (m+1)*8)), AllGathers the 1024 (s,t) pairs, then every core
computes the full (tiny) loss redundantly.
"""

import os
import sys

import numpy as np

if "/opt/trn_rl_repo" not in sys.path:
    sys.path.insert(0, "/opt/trn_rl_repo")

N_DOCS = 10_000_000
N = 8192
P = 128
NJ = N // P  # 64 columns
NCORES = 8
NJL = NJ // NCORES  # 8 columns per core in the sharded variant

_BUILD_CACHE = {}


def _consts_np() -> np.ndarray:
    """[128, 200] f32 constant block; see _build for the sub-views."""
    C = np.zeros((P, 200), np.float32)
    pp = np.arange(P)
    # Lincl[p', p] = 1 if p' >= p   (lhsT for within-column inclusive suffix)
    C[:, 0:P] = (pp[:, None] >= pp[None, :]).astype(np.float32)
    # U64strict[j', j] = 1 if j' > j  (rhs for column-tail broadcast matmul)
    jj = np.arange(NJ)
    C[0:NJ, P : P + NJ] = (jj[:, None] > jj[None, :]).astype(np.float32)
    # ones column
    C[:, 192] = 1.0
    # SgT[u, w] = -0.5 * sign(w - u): g = SgT.T @ c gives per-value row weight
    vv = np.arange(5)
    C[0:5, 193:198] = -0.5 * np.sign(vv[None, :] - vv[:, None]).astype(np.float32)
    return C


def _compute_loss(nc, pool, psum, cst_sb, s_ap, t_ap, loss_dram, dbg=None):
    """Given s and t [128, 64] f32 views in SBUF, write loss [1] to DRAM."""
    import concourse.mybir as mybir

    f32 = mybir.dt.float32
    Lincl = cst_sb[:, 0:P]
    U64 = cst_sb[0:NJ, P : P + NJ]
    ones = cst_sb[:, 192:193]
    sgT = cst_sb[0:5, 193:198]

    # PE instructions are hw-decoded and support only one semaphore wait.
    # Consume the constants-DMA wait with a throwaway matmul so every real
    # matmul below only ever waits on the (single) DVE semaphore.
    warm_ps = psum.tile([1, 1], f32, space="PSUM")
    nc.tensor.matmul(warm_ps[:], lhsT=ones, rhs=ones, start=True, stop=True)

    # 5 one-hot masks of t, side by side
    M = pool.tile([P, 5 * NJ], f32)
    for v in range(5):
        nc.vector.tensor_scalar(
            M[:, v * NJ : (v + 1) * NJ], t_ap, float(v), None, mybir.AluOpType.is_equal
        )
    # per-partition histogram (free-dim reduce), then global counts via matmul
    R = pool.tile([P, 8], f32)
    for v in range(5):
        nc.vector.reduce_sum(
            R[:, v : v + 1], M[:, v * NJ : (v + 1) * NJ], axis=mybir.AxisListType.X
        )
    c_ps = psum.tile([5, 1], f32, space="PSUM")
    nc.tensor.matmul(c_ps[:], lhsT=R[:, 0:5], rhs=ones, start=True, stop=True)
    c_sb = pool.tile([5, 1], f32)
    nc.vector.tensor_copy(c_sb[:], c_ps[:])
    # g[w] = -0.5 * sum_u c_u * sign(w - u)
    g_ps = psum.tile([5, 1], f32, space="PSUM")
    nc.tensor.matmul(g_ps[:], lhsT=sgT, rhs=c_sb[:], start=True, stop=True)
    g_sb = pool.tile([5, 1], f32)
    nc.vector.tensor_copy(g_sb[:], g_ps[:])

    # inclusive suffix sums T[p, j] = sum_{p'>=p} s[p', j] + sum_{j'>j} colsum[j']
    Ct_ps = psum.tile([NJ, 1], f32, space="PSUM")
    nc.tensor.matmul(Ct_ps[:], lhsT=s_ap, rhs=ones, start=True, stop=True)
    ct_sb = pool.tile([NJ, 1], f32)
    nc.vector.tensor_copy(ct_sb[:], Ct_ps[:])
    ctb = pool.tile([NJ, P], f32)
    nc.vector.tensor_copy(ctb[:], ct_sb[:].to_broadcast([NJ, P]))
    W_ps = psum.tile([P, NJ], f32, space="PSUM")
    nc.tensor.matmul(W_ps[:], lhsT=Lincl, rhs=s_ap, start=True, stop=False)
    nc.tensor.matmul(W_ps[:], lhsT=ctb[:], rhs=U64, start=False, stop=True)
    T_sb = pool.tile([P, NJ], f32)
    nc.vector.tensor_copy(T_sb[:], W_ps[:])

    # h_v = sum_k [t_k == v] * T_k ; loss = sum_v g_v h_v
    H = pool.tile([P, 8], f32)
    for v in range(5):
        mv = M[:, v * NJ : (v + 1) * NJ]
        nc.vector.tensor_tensor(mv, mv, T_sb[:], op=mybir.AluOpType.mult)
        nc.vector.reduce_sum(H[:, v : v + 1], mv, axis=mybir.AxisListType.X)
    h_ps = psum.tile([5, 1], f32, space="PSUM")
    nc.tensor.matmul(h_ps[:], lhsT=H[:, 0:5], rhs=ones, start=True, stop=True)
    h_sb = pool.tile([5, 1], f32)
    nc.vector.tensor_copy(h_sb[:], h_ps[:])
    gh_sb = pool.tile([5, 1], f32)
    nc.vector.tensor_tensor(gh_sb[:], g_sb[:], h_sb[:], op=mybir.AluOpType.mult)
    l_ps = psum.tile([1, 1], f32, space="PSUM")
    nc.tensor.matmul(l_ps[:], lhsT=gh_sb[:], rhs=ones[0:5, :], start=True, stop=True)
    out_sb = pool.tile([1, 1], f32)
    nc.vector.tensor_copy(out_sb[:], l_ps[:])
    nc.sync.dma_start(loss_dram[:], out_sb[0:1, 0:1])

    if dbg is not None:
        dbg_s, dbg_t, dbg_T, dbg_cgh = dbg
        s_cp = pool.tile([P, NJ], f32)
        nc.vector.tensor_copy(s_cp[:], s_ap)
        nc.sync.dma_start(dbg_s[:, :], s_cp[:])
        t_cp = pool.tile([P, NJ], f32)
        nc.vector.tensor_copy(t_cp[:], t_ap)
        nc.sync.dma_start(dbg_t[:, :], t_cp[:])
        nc.sync.dma_start(dbg_T[:, :], T_sb[:])
        cgh = pool.tile([5, 3], f32)
        nc.vector.tensor_copy(cgh[:, 0:1], c_sb[:])
        nc.vector.tensor_copy(cgh[:, 1:2], g_sb[:])
        nc.vector.tensor_copy(cgh[:, 2:3], h_sb[:])
        nc.sync.dma_start(dbg_cgh[:, :], cgh[:])


def _build(sharded: bool):
    import concourse.bacc as bacc
    import concourse.bass as bass
    import concourse.mybir as mybir
    from concourse.tile import TileContext

    key = (
        "sharded" if sharded else "replicated",
        os.environ.get("RANKNET_DEBUG", "0"),
    )
    if key in _BUILD_CACHE:
        return _BUILD_CACHE[key]

    f32 = mybir.dt.float32
    nc = bacc.Bacc("TRN2", target_bir_lowering=False, debug=True)
    njl = NJL if sharded else NJ

    tbl = nc.declare_dram_parameter("tbl", [N_DOCS, 2], f32, isOutput=False)
    idx = nc.declare_dram_parameter("idx", [P, njl], mybir.dt.int32, isOutput=False)
    cst = nc.declare_dram_parameter("cst", [P, 200], f32, isOutput=False)
    loss = nc.declare_dram_parameter("loss", [1], f32, isOutput=True)
    debug = bool(int(os.environ.get("RANKNET_DEBUG", "0")))
    dbg = None
    if debug:
        dbg = (
            nc.declare_dram_parameter("dbg_s", [P, NJ], f32, isOutput=True),
            nc.declare_dram_parameter("dbg_t", [P, NJ], f32, isOutput=True),
            nc.declare_dram_parameter("dbg_T", [P, NJ], f32, isOutput=True),
            nc.declare_dram_parameter("dbg_cgh", [5, 3], f32, isOutput=True),
        )

    with TileContext(nc) as tc:
        with (
            tc.tile_pool(name="sbuf", bufs=1) as pool,
            tc.tile_pool(name="psum", bufs=1, space="PSUM") as psum,
            tc.tile_pool(name="dram", bufs=1, space="DRAM") as dram,
        ):
            cst_sb = pool.tile([P, 200], f32)
            nc.sync.dma_start(cst_sb[:], cst[:])
            idx_sb = pool.tile([P, njl], mybir.dt.int32)
            nc.sync.dma_start(idx_sb[:], idx[:])

            # gather (s, t) pairs: out[p, 2j] = s_k, out[p, 2j+1] = t_k.
            # HW indirect DMA consumes ONE index per partition per instruction
            # (fetching one contiguous table row into that partition), so each
            # column of indices is its own gather.
            st_sb = pool.tile([P, 2 * njl], f32)
            for j in range(njl):
                nc.gpsimd.indirect_dma_start(
                    out=st_sb[:, 2 * j : 2 * j + 2],
                    out_offset=None,
                    in_=tbl[:, :],
                    in_offset=bass.IndirectOffsetOnAxis(
                        ap=idx_sb[:, j : j + 1], axis=0
                    ),
                )

            if sharded:
                cc_in = dram.tile([P * 2 * NJL], f32)  # 2048
                nc.sync.dma_start(cc_in[:], st_sb[:])
                cc_out = dram.tile([NCORES, P * 2 * NJL], f32)
                nc.gpsimd.collective_compute(
                    "AllGather",
                    mybir.AluOpType.bypass,
                    replica_groups=[list(range(NCORES))],
                    ins=[cc_in[:].opt()],
                    outs=[cc_out[:].opt()],
                )
                # reassemble the full interleaved [128, 128] tile:
                # st_full[p, (m*NJL+jl)*2+c] = cc_out[m, p*2*NJL + 2*jl + c]
                st_full = pool.tile([P, 2 * NJ], f32)
                base = cc_out[:]
                src = bass.AP(
                    base.tensor,
                    base.offset,
                    [[2 * NJL, P], [P * 2 * NJL, NCORES], [1, 2 * NJL]],
                )
                dst = st_full[:, :].rearrange("p (m x) -> p m x", m=NCORES)
                nc.sync.dma_start(dst, src)
                s_sb = pool.tile([P, NJ], f32)
                nc.vector.tensor_copy(s_sb[:], st_full[:, 0 : 2 * NJ : 2])
                s_ap = s_sb[:, :]
                t_ap = st_full[:, 1 : 2 * NJ : 2]
            else:
                s_sb = pool.tile([P, NJ], f32)
                nc.vector.tensor_copy(s_sb[:], st_sb[:, 0 : 2 * NJ : 2])
                s_ap = s_sb[:, :]
                t_ap = st_sb[:, 1 : 2 * NJ : 2]

            _compute_loss(nc, pool, psum, cst_sb, s_ap, t_ap, loss, dbg=dbg)

    nc.compile()
    _BUILD_CACHE[key] = nc
    return nc


def _make_in_maps(tbl, cst, rd, sharded: bool):
    if sharded:
        in_maps = []
        for m in range(NCORES):
            sl = rd[m * (N // NCORES) : (m + 1) * (N // NCORES)]
            in_maps.append(
                {"tbl": tbl, "idx": np.ascontiguousarray(sl.reshape(NJL, P).T), "cst": cst}
            )
        return in_maps
    im = {"tbl": tbl, "idx": np.ascontiguousarray(rd.reshape(NJ, P).T), "cst": cst}
    return [im for _ in range(NCORES)]


def _prep_inputs(scores, target, random_docs):
    scores = np.asarray(scores, dtype=np.float32).reshape(N_DOCS)
    target = np.asarray(target)
    rd = np.asarray(random_docs, dtype=np.int32)
    tbl = np.empty((N_DOCS, 2), np.float32)
    tbl[:, 0] = scores
    tbl[:, 1] = target
    return tbl, _consts_np(), rd


def run(scores, target, random_docs, trace=False, **spmd_kwargs):
    from concourse.bass_utils import run_bass_kernel_spmd

    sharded = os.environ.get("RANKNET_VARIANT", "sharded") == "sharded"
    tbl, cst, rd = _prep_inputs(scores, target, random_docs)
    nc = _build(sharded)
    in_maps = _make_in_maps(tbl, cst, rd, sharded)
    res = run_bass_kernel_spmd(
        nc, in_maps, list(range(NCORES)), trace=trace, **spmd_kwargs
    )
    out = np.asarray(res.results[0]["loss"], np.float32).reshape(1)
    return out, res


def kernel(scores, target, random_docs):
    out, _ = run(scores, target, random_docs, trace=False)
    return out
